# revision 40
# baseline (speedup 1.0000x reference)
"""Trainium2 Bass kernel for the attention-pooling module.

Reference math (B=32, N=2048, D=512, K=256):
    vIp   = vI @ Wi                                   [B,N,K]
    vQp   = vQ @ Wq + bq                              [B,K]
    ha    = leaky_relu(vIp + vQp[:,None,:], 0.01)     [B,N,K]
    scores= ha @ Wp[:,0] + bp                         [B,N]   (bp cancels in softmax)
    pi    = softmax(scores, -1)                       [B,N]
    out   = einsum("bn,bnk->bk", pi, vIp) + vQp       [B,K]

Device (per core, 4 batches, data-parallel over B) computes only what needs
the bulk tensor; vI streams twice in fp8 (d-major for scores, n-major for
the attention reduction -- the DVE has no fast fused mul-reduce, so both
contractions run on the PE):
    scores path: vpT = (16*Wi)^T @ vIT   (fp8 DoubleRow matmuls, K on partitions)
                 ha  = ACT Prelu(vpT/16 + vqpT) -> fp8   (Prelu == leaky relu,
                       and unlike Lrelu it shares an ACT table set with Exp,
                       avoiding 1.28us table reloads on every batch)
                 scores = (8*Wp)^T @ ha  (PE, M=1)
                 e-chain: scores -> SBUF bf16 -> [16,128] gather -> PE
                 transpose -> exp on [128,16] (accum_out = Z partial sums)
    attention:   u = e @ vI  (8 fp8 DoubleRow matmuls vs the n-major copy)
Host computes the precision-critical small math exactly in fp32:
    vQp = vQ@Wq + bq (also shipped as the device-side Prelu bias),
    att = (u @ Wi)/Z,  out = att + vQp.

Schedule notes (all hard-won against engine-FIFO head-of-line blocking):
  - 14 warm-up matmuls on a 0.5-filled tile spin the PE HAM clock-gate to
    2.4 GHz through the DMA-paced start. All-zero warm-ups do NOT register
    (the activity monitor tracks data toggling); short warm-ups (<3.4 us
    sustained) never trigger the un-throttle window.
  - DRAM layouts keep each partition's data contiguous (128-descriptor
    DMAs, ~0.6us HWDGE issue each); vit0 lands in (sp,cc) pieces so the
    first matmul starts after ~0.7 MB; all scores-path tiles stream before
    any vnat. Single Sync-ring issue: splitting issues onto the Scalar
    HWDGE ring stalls the ACT FIFO and costs more than it saves.
  - The xbar DMA-transpose was replaced by a PE transpose: it serializes
    against every in-flight DMA and stalled each e-chain behind the whole
    input stream.
  - Each supertile's scores matmuls are emitted one vp-block late, each
    batch's e-chain two blocks late, and the u phases run at the end, so
    the PE FIFO (strictly in-order) never parks on an ACT/DVE/DMA result.
  - PSUM: vp ring and scores ring and attention ring are separate pools;
    a shared ring couples u-matmul PSUM writes to DVE reads of recycled
    score banks (bank-aware serialization stalled every u matmul ~600ns).
"""

import os
import sys

sys.path.insert(0, "/opt/trn_rl_repo")

import numpy as np
import ml_dtypes

from concourse import bass, bacc, tile, mybir
from concourse.bass_utils import run_bass_kernel_spmd

dt = mybir.dt
F32, BF16, FP8 = dt.float32, dt.bfloat16, dt.float8e4
AF = mybir.ActivationFunctionType
ALU = mybir.AluOpType

B, N, D, K = 32, 2048, 512, 256
NCORES = 8
BLOC = B // NCORES           # 4 batches per core
SUP = 512                    # scores supertile (PSUM-bank limited)
DC = D // 128                # 4
KC = K // 128                # 2
NEG = 0.01
NWARM = 14                   # ~6 us of warm-up matmuls: HAM needs >3.4 us
                             # sustained, and the stream stays DMA-paced (too
                             # sparse to keep it warm) until ~14 us


def build_nc():
    nc = bacc.Bacc("TRN2", target_bir_lowering=False, debug=False)

    # vit: d-on-partitions layout; [b, p, sp, cc, i, n1024], d = cc*256+i*128+p,
    # n = sp*1024 + n1024. Each partition row is 8 KiB contiguous in DRAM.
    vit_d = nc.dram_tensor("vit", [BLOC, 128, 2, 2, 2, 1024], FP8, kind="ExternalInput")
    # vnat: n-on-partitions layout; [b, pn, t, d], n = t*128 + pn.
    vnat_d = nc.dram_tensor("vnat", [BLOC, 128, 16, D], FP8, kind="ExternalInput")
    wi8_d = nc.dram_tensor("wi8", [128, 2, 2, K], FP8, kind="ExternalInput")
    # pkw: vqpt(8: host-exact vQp^T, the Lrelu bias) | wp_dr(2x16, wp*8 in
    # col j=0) | id16(16) -- all f32, 28 KiB
    pkw_d = nc.dram_tensor("pkw", [128, 56], F32, kind="ExternalInput")
    u_d = nc.dram_tensor("u", [1, BLOC, D], F32, kind="ExternalOutput")
    zp_d = nc.dram_tensor("zp", [128, BLOC], F32, kind="ExternalOutput")

    with tile.TileContext(nc) as tc:
        with (
            tc.tile_pool(name="const", bufs=1) as cpool,
            tc.tile_pool(name="stream", bufs=4) as spool,
            tc.tile_pool(name="work", bufs=3) as wpool,
            tc.tile_pool(name="pmm", bufs=2, space=bass.MemorySpace.PSUM) as pmm,
            tc.tile_pool(name="psm", bufs=2, space=bass.MemorySpace.PSUM) as psm,
            tc.tile_pool(name="patt", bufs=2, space=bass.MemorySpace.PSUM) as patt,
        ):
            pkw_sb = cpool.tile([128, 56], F32, tag="pkw")
            wi8_sb = cpool.tile([128, 2, 2, K], FP8, tag="wi8")
            junk = cpool.tile([128, SUP], FP8, tag="junk")
            wp8 = cpool.tile([128, 2, 16], FP8, tag="wp8")
            id16b = cpool.tile([128, 16], BF16, tag="id16b")
            u_sb = cpool.tile([1, BLOC, D], F32, tag="usb")
            zp_sb = cpool.tile([128, BLOC], F32, tag="zpsb")

            vit_tiles = [
                spool.tile([128, 2, 2, 2, 1024], FP8, tag="vit", name=f"vit{b}")
                for b in range(BLOC)
            ]
            vnat_tiles = [
                spool.tile([128, 16, D], FP8, tag="vnat", name=f"vnat{b}")
                for b in range(BLOC)
            ]

            # ---- input DMAs ----------------------------------------------
            # vit0 lands in (sp, cc) pieces so the first matmul can start
            # after ~0.7 MB has streamed. All vit (scores-path) tiles stream
            # before any vnat: the scores phases are the PE critical path,
            # the u phases run at the end. (Splitting issues across the
            # Scalar HWDGE ring regressed: it blocks the ACT FIFO.)
            nc.sync.dma_start(out=pkw_sb[:], in_=pkw_d[:])
            nc.sync.dma_start(out=vit_tiles[0][:, 0, 0], in_=vit_d[0][:, 0, 0])
            nc.sync.dma_start(out=wi8_sb[:], in_=wi8_d[:])
            nc.sync.dma_start(out=vit_tiles[0][:, 0, 1], in_=vit_d[0][:, 0, 1])
            nc.sync.dma_start(out=vit_tiles[0][:, 1], in_=vit_d[0][:, 1])
            nc.sync.dma_start(out=vit_tiles[1][:], in_=vit_d[1])
            nc.sync.dma_start(out=vit_tiles[2][:], in_=vit_d[2])
            nc.sync.dma_start(out=vit_tiles[3][:], in_=vit_d[3])
            nc.sync.dma_start(out=vnat_tiles[0][:], in_=vnat_d[0])
            nc.sync.dma_start(out=vnat_tiles[1][:], in_=vnat_d[1])
            nc.sync.dma_start(out=vnat_tiles[2][:], in_=vnat_d[2])
            nc.sync.dma_start(out=vnat_tiles[3][:], in_=vnat_d[3])

            # ---- PE warm-up: HAM un-throttles after ~3.4 us of activity.
            # Non-zero data: the activity monitor tracks toggling, all-zero
            # matmuls do not register (warm-up had no effect with 0s).
            nc.gpsimd.memset(junk[:], 0.5)
            for w in range(NWARM):
                wps = pmm.tile([128, SUP], F32, tag="vp", name=f"warm{w}")
                nc.tensor.matmul(
                    wps[:], junk[:, 0:128], junk[:], start=True, stop=True
                )
            # preload the ACT tables (Lrelu/Exp/Copy) while the DMAs stream;
            # a lazy mid-kernel ACT_TABLE_LOAD costs 1.28us on the e-chain
            actw = wpool.tile([128, 16], BF16, tag="actw")
            nc.scalar.activation(actw[:], junk[:, 0:16], AF.Prelu, alpha=NEG)
            nc.scalar.activation(actw[:], junk[:, 0:16], AF.Exp)
            nc.scalar.copy(actw[:], junk[:, 0:16])  # Copy shares the exp table set

            # ---- unpack pkw: host-exact vqp^T bias, wp8, bf16 identity ----
            vqpt_sb = pkw_sb[:, 0:8].rearrange("p (kc b) -> p kc b", kc=KC)
            wp32 = pkw_sb[:, 8:40].rearrange("p (i j) -> p i j", i=2)
            nc.vector.tensor_copy(wp8[:], wp32[:])
            nc.vector.tensor_copy(id16b[:], pkw_sb[:, 40:56])

            scrows = [None] * BLOC
            has = {}

            def phase_vp(b, sp):
                # 8 DoubleRow matmuls into PSUM + 2 Lrelus -> ha fp8
                if sp == 0:
                    scrows[b] = wpool.tile([1, N], BF16, tag="scrow", name=f"scrow{b}")
                vit = vit_tiles[b]
                ha = wpool.tile([128, KC, 1024], FP8, tag="ha", name=f"ha{b}{sp}")
                has[(b, sp)] = ha
                for kc in range(KC):
                    vp = pmm.tile([128, 1024], F32, tag="vp", name=f"vp{b}{sp}{kc}")
                    for cc in range(2):
                        for h in range(2):
                            nc.tensor.matmul(
                                vp[:, h * SUP : (h + 1) * SUP],
                                wi8_sb[:, cc, :, kc * 128 : (kc + 1) * 128],
                                vit[:, sp, cc, :, h * SUP : (h + 1) * SUP],
                                perf_mode=mybir.MatmulPerfMode.DoubleRow,
                                start=(cc == 0),
                                stop=(cc == 1),
                            )
                    # Wi is host-scaled x16 into fp8 normal range; ACT
                    # de-scales for free: ha = lrelu(vp/16 + vqp)
                    nc.scalar.activation(
                        ha[:, kc, :], vp[:], AF.Prelu,
                        bias=vqpt_sb[:, kc, b : b + 1], scale=1.0 / 16, alpha=NEG,
                    )

            def phase_scmm(b, sp):
                # deferred a full vp-block so the PE FIFO never waits on the
                # second Lrelu (was a ~0.7us stall at every supertile boundary)
                ha, scrow = has[(b, sp)], scrows[b]
                scps = [
                    psm.tile([1, SUP], F32, tag="small", name=f"scp{b}_{sp}_{h}")
                    for h in range(2)
                ]
                for h in range(2):
                    nc.tensor.matmul(
                        scps[h][:], wp8[:, :, 0:1],
                        ha[:, :, h * SUP : (h + 1) * SUP],
                        perf_mode=mybir.MatmulPerfMode.DoubleRow,
                        start=True, stop=True,
                    )
                for h in range(2):
                    n0 = sp * 1024 + h * SUP
                    nc.vector.tensor_copy(scrow[0:1, n0 : n0 + SUP], scps[h][:])

            e_cols = [None] * BLOC

            def phase_echain(b):
                scrow = scrows[b]
                # scores -> [16,128] (small SBUF-SBUF gather) -> PE transpose
                # -> PSUM [128,16], exp reads PSUM. (The xbar dma transpose
                # serializes against every in-flight DMA and stalled the
                # e-chain behind the whole input stream.)
                s16 = wpool.tile([16, 128], BF16, tag="s16", name=f"s16_{b}")
                nc.sync.dma_start(
                    out=s16[:], in_=scrow[0:1, :].rearrange("o (t p) -> o t p", p=128)
                )
                s_ps = patt.tile([128, 16], BF16, tag="att", name=f"sps{b}")
                nc.tensor.transpose(s_ps[:], s16[:], id16b[0:16, :])

                # [128, 2, 16]: pair partner at +16B so the DoubleRow
                # lhsT AP satisfies the 16B-step ISA constraint.
                # Wp is host-scaled x8 (fp8 range); exp de-scales for free.
                e_col = wpool.tile([128, 2, 16], FP8, tag="ecol", name=f"ecol{b}")
                e_cols[b] = e_col
                nc.scalar.activation(
                    e_col[:].rearrange("p i j -> p j i")[:, 0:8, :],
                    s_ps[:].rearrange("p (j i) -> p j i", i=2),
                    AF.Exp, scale=1.0 / 8, accum_out=zp_sb[:, b : b + 1],
                )

            def phase_u(b):
                vnat, e_col = vnat_tiles[b], e_cols[b]
                # u = e @ vI on the PE: 8 accumulating fp8 DoubleRow matmuls
                ups = patt.tile([1, D], F32, tag="att", name=f"ups{b}")
                NT = N // 128
                for t in range(0, NT, 2):
                    nc.tensor.matmul(
                        ups[:],
                        e_col[:, :, t // 2 : t // 2 + 1],  # pair stride 16B
                        vnat[:, t : t + 2, :],
                        perf_mode=mybir.MatmulPerfMode.DoubleRow,
                        start=(t == 0),
                        stop=(t == NT - 2),
                    )
                nc.vector.tensor_copy(u_sb[:, b, :], ups[:])

            # Pipeline: vp blocks run back-to-back on the PE; each supertile's
            # scores matmuls and each batch's e-chain (DVE copies / s16
            # gather / PE transpose / ACT exp) are emitted one vp-block late
            # so the PE FIFO never waits on an ACT result; the u matmul
            # phases are deferred to the end so no u phase stalls on its e.
            phase_vp(0, 0)
            phase_vp(0, 1)
            phase_scmm(0, 0)
            phase_vp(1, 0)
            phase_scmm(0, 1)
            phase_vp(1, 1)
            phase_scmm(1, 0)
            phase_vp(2, 0)
            phase_echain(0)
            phase_scmm(1, 1)
            phase_vp(2, 1)
            phase_scmm(2, 0)
            phase_vp(3, 0)
            phase_echain(1)
            phase_scmm(2, 1)
            phase_vp(3, 1)
            phase_scmm(3, 0)
            phase_scmm(3, 1)
            phase_echain(2)
            phase_u(0)
            phase_echain(3)
            phase_u(1)
            phase_u(2)
            phase_u(3)

            nc.sync.dma_start(out=u_d[:], in_=u_sb[:])
            nc.sync.dma_start(out=zp_d[:], in_=zp_sb[:])

    nc.compile()
    return nc


_NC = None


def _get_nc():
    global _NC
    if _NC is None:
        _NC = build_nc()
    return _NC


def kernel(vI, vQ, Wi, Wq, bq, Wp, bp, **_unused):
    vI = np.asarray(vI, dtype=np.float32)
    vQ = np.asarray(vQ, dtype=np.float32)
    Wi = np.asarray(Wi, dtype=np.float32)
    Wq = np.asarray(Wq, dtype=np.float32)
    bq = np.asarray(bq, dtype=np.float32)
    Wp = np.asarray(Wp, dtype=np.float32)
    # bp shifts every score equally -> cancels in softmax; ignored (and 0).

    bf = ml_dtypes.bfloat16
    f8 = ml_dtypes.float8_e4m3

    vi8 = vI.astype(f8)
    # vit: [B, p, sp, cc, i, n1024]; d = cc*256 + i*128 + p, n = sp*1024 + n'
    vit = np.ascontiguousarray(
        vi8.reshape(B, 2, 1024, 2, 2, 128).transpose(0, 5, 1, 3, 4, 2)
    )
    # vnat: [B, pn, t, d]; n = t*128 + pn
    vnat = np.ascontiguousarray(
        vi8.reshape(B, 16, 128, D).transpose(0, 2, 1, 3)
    )
    wi8 = np.ascontiguousarray(
        (Wi * 16.0).reshape(2, 2, 128, K).transpose(2, 0, 1, 3)
    ).astype(f8)                                                  # [128,cc,i,K]
    vQp = vQ @ Wq + bq                                            # exact fp32
    wp_pad = np.zeros((128, 2, 16), np.float32)
    wp_pad[:, :, 0] = Wp[:, 0].reshape(KC, 128).T * 8.0
    id16 = np.zeros((128, 16), np.float32)
    id16[0:16, :] = np.eye(16, dtype=np.float32)

    def pkw_for(core):
        vqpc = vQp[core * BLOC : (core + 1) * BLOC]               # [BLOC, K]
        # vqpt[p, kc, b] = vQp[b, kc*128 + p]
        vqpt = vqpc.reshape(BLOC, KC, 128).transpose(2, 1, 0)     # [128,KC,BLOC]
        return np.ascontiguousarray(
            np.concatenate(
                [vqpt.reshape(128, KC * BLOC), wp_pad.reshape(128, 32), id16],
                axis=1,
            ).astype(np.float32)
        )

    in_maps = []
    for c in range(NCORES):
        in_maps.append(
            {
                "vit": vit[c * BLOC : (c + 1) * BLOC],
                "vnat": vnat[c * BLOC : (c + 1) * BLOC],
                "wi8": wi8,
                "pkw": pkw_for(c),
            }
        )

    nc = _get_nc()
    res = run_bass_kernel_spmd(
        nc, in_maps, list(range(NCORES)),
        trace=bool(int(os.environ.get("KERNEL_TRACE", "0"))),
        tmpdir=globals().get("TRACE_TMPDIR"),
    )
    kernel.last_results = res

    # host-side exact fp32 finish: out = (u @ Wi)/Z + vQp
    out = np.empty((B, K), np.float32)
    for c in range(NCORES):
        u = np.asarray(res.results[c]["u"], np.float32)[0]        # [BLOC, D]
        zp = np.asarray(res.results[c]["zp"], np.float32)         # [128, BLOC]
        Z = zp.sum(axis=0)                                        # [BLOC]
        att = (u @ Wi) / Z[:, None]                               # [BLOC, K]
        out[c * BLOC : (c + 1) * BLOC] = att + vQp[c * BLOC : (c + 1) * BLOC]
    return out
